# revision 1
# baseline (speedup 1.0000x reference)
"""Performer head kernel for Trainium2 (8 NeuronCores, data-parallel over batch).

Key structural insight: the model output is CLS pooling of token 0 and the
attention is causal, so token 0 attends only to itself.  At token 0 the
chunked causal linear attention reduces exactly to v[0] (num = attn00*v0,
den = attn00), independent of the FAVOR+ features.  The head split/concat is
the identity on the feature dimension, so each layer collapses to

    h += LN1(h) @ (wv @ wo) + (bias terms)
    h += gelu_tanh(LN2(h) @ w1 + b1) @ w2 + b2

and the output is h @ w_cls + b_cls.  This is an exact algebraic
simplification of the reference (verified to ~1e-7 relative error).

Sharding: data-parallel over batch; each of the 8 cores processes 2 of the
16 batch rows.  Weights are replicated; per-core inputs differ only in the
two token-0 embeddings.
"""

import numpy as np

import concourse.bass as bass
import concourse.bacc as bacc
import concourse.tile as tile
from concourse import mybir
from concourse.bass_utils import run_bass_kernel_spmd

F32 = mybir.dt.float32
AX = mybir.AxisListType
OP = mybir.AluOpType
AF = mybir.ActivationFunctionType

# Model constants (hardcoded from the problem spec).
B, N, D, HEADS, DH, M, L, C = 16, 4096, 256, 4, 64, 128, 4, 10
FF = 4 * D          # 1024
NKB = D // 128      # 2 feature blocks
NFB = FF // 128     # 8 hidden blocks
N_CORES = 8
B_LOC = B // N_CORES  # 2 tokens (batch rows) per core
LN_EPS = 1e-5

_CACHE = {}


def _build_nc():
    nc = bacc.Bacc(None, target_bir_lowering=False)

    h0_d = nc.dram_tensor("h0", [B_LOC, D], F32, kind="ExternalInput")
    wvo_d = nc.dram_tensor("wvo", [L, 128, NKB, D], F32, kind="ExternalInput")
    bvo_d = nc.dram_tensor("bvo", [L, 1, D], F32, kind="ExternalInput")
    w1_d = nc.dram_tensor("w1b", [L, 128, NKB, NFB, 128], F32, kind="ExternalInput")
    b1_d = nc.dram_tensor("b1r", [L, 1, NFB, 128], F32, kind="ExternalInput")
    w2_d = nc.dram_tensor("w2b", [L, 128, NFB, D], F32, kind="ExternalInput")
    b2_d = nc.dram_tensor("b2r", [L, 1, D], F32, kind="ExternalInput")
    wcls_d = nc.dram_tensor("wclsb", [128, NKB, C], F32, kind="ExternalInput")
    bcls_d = nc.dram_tensor("bclsr", [1, C], F32, kind="ExternalInput")
    id2_d = nc.dram_tensor("id2", [B_LOC, B_LOC], F32, kind="ExternalInput")
    ones1_d = nc.dram_tensor("ones1", [1, B_LOC], F32, kind="ExternalInput")
    out_d = nc.dram_tensor("out", [B_LOC, C], F32, kind="ExternalOutput")

    with tile.TileContext(nc) as tc:
        with tc.tile_pool(name="wts", bufs=1) as wts, \
             tc.tile_pool(name="act", bufs=2) as act, \
             tc.tile_pool(name="ln", bufs=2) as lnp, \
             tc.tile_pool(name="ps", bufs=2, space="PSUM") as ps, \
             tc.tile_pool(name="psg", bufs=2, space="PSUM") as psg:

            # --- load all weights up front (layer-major order) ---
            id2_sb = wts.tile([B_LOC, B_LOC], F32)
            nc.sync.dma_start(out=id2_sb, in_=id2_d[:, :])
            ones1_sb = wts.tile([1, B_LOC], F32)
            nc.sync.dma_start(out=ones1_sb, in_=ones1_d[:, :])
            eps_col = wts.tile([B_LOC, 1], F32)
            nc.vector.memset(eps_col, LN_EPS)

            wvo_sb, bvo_sb, w1_sb, b1_sb, w2_sb, b2_sb = [], [], [], [], [], []
            for l in range(L):
                t = wts.tile([128, NKB, D], F32, name=f"wvo{l}")
                nc.sync.dma_start(out=t, in_=wvo_d[l])
                wvo_sb.append(t)
                t = wts.tile([1, D], F32, name=f"bvo{l}")
                nc.sync.dma_start(out=t, in_=bvo_d[l])
                bvo_sb.append(t)
                t = wts.tile([128, NKB, NFB, 128], F32, name=f"w1b{l}")
                nc.sync.dma_start(out=t, in_=w1_d[l])
                w1_sb.append(t)
                t = wts.tile([1, NFB, 128], F32, name=f"b1r{l}")
                nc.sync.dma_start(out=t, in_=b1_d[l])
                b1_sb.append(t)
                t = wts.tile([128, NFB, D], F32, name=f"w2b{l}")
                nc.sync.dma_start(out=t, in_=w2_d[l])
                w2_sb.append(t)
                t = wts.tile([1, D], F32, name=f"b2r{l}")
                nc.sync.dma_start(out=t, in_=b2_d[l])
                b2_sb.append(t)
            wcls_sb = wts.tile([128, NKB, C], F32)
            nc.sync.dma_start(out=wcls_sb, in_=wcls_d[:, :, :])
            bcls_sb = wts.tile([1, C], F32)
            nc.sync.dma_start(out=bcls_sb, in_=bcls_d[:, :])

            h_sb = act.tile([B_LOC, D], F32, bufs=1)
            nc.sync.dma_start(out=h_sb, in_=h0_d[:, :])

            def ln_transpose(tag):
                """LayerNorm h (no scale/bias: folded into weights), then
                transpose to feature-major zT [128, NKB, B_LOC]."""
                stats = lnp.tile([B_LOC, 6], F32, name=f"stats_{tag}", tag="stats")
                nc.vector.bn_stats(out=stats, in_=h_sb[:, :])
                mv = lnp.tile([B_LOC, 2], F32, name=f"mv_{tag}", tag="mv")
                nc.vector.bn_aggr(out=mv, in_=stats)
                sd = lnp.tile([B_LOC, 1], F32, name=f"sd_{tag}", tag="sd")
                nc.scalar.activation(out=sd, in_=mv[:, 1:2], func=AF.Sqrt,
                                     bias=eps_col, scale=1.0)
                rstd = lnp.tile([B_LOC, 1], F32, name=f"rstd_{tag}", tag="rstd")
                nc.vector.reciprocal(out=rstd, in_=sd)
                z = lnp.tile([B_LOC, D], F32, name=f"z_{tag}", tag="z")
                nc.vector.tensor_scalar(out=z, in0=h_sb[:, :],
                                        scalar1=mv[:, 0:1], scalar2=rstd,
                                        op0=OP.subtract, op1=OP.mult)
                zT = lnp.tile([128, NKB, B_LOC], F32, name=f"zT_{tag}", tag="zT")
                for kb in range(NKB):
                    pt = ps.tile([128, B_LOC], F32, name=f"pt_{tag}{kb}", tag="pt")
                    nc.tensor.transpose(pt[:, :], z[:, kb * 128:(kb + 1) * 128],
                                        id2_sb[:, :])
                    nc.scalar.copy(out=zT[:, kb, :], in_=pt[:, :])
                return zT

            for l in range(L):
                # --- attention block (token 0): h += z @ Wvo + Bvo ---
                zT = ln_transpose(f"a{l}")
                po = ps.tile([B_LOC, D], F32, tag="po", name=f"po_a{l}")
                for kb in range(NKB):
                    nc.tensor.matmul(po[:, :], zT[:, kb, :], wvo_sb[l][:, kb, :],
                                     start=(kb == 0), stop=False)
                nc.tensor.matmul(po[:, :], ones1_sb[:, :], bvo_sb[l][:, :],
                                 start=False, stop=True)
                nc.vector.tensor_add(out=h_sb[:, :], in0=h_sb[:, :], in1=po[:, :])

                # --- MLP block: h += gelu(z2 @ w1 + b1) @ w2 + b2 ---
                z2T = ln_transpose(f"m{l}")
                pg = psg.tile([128, NFB, B_LOC], F32, tag="pg", name=f"pg{l}")
                for mb in range(NFB):
                    for kb in range(NKB):
                        nc.tensor.matmul(pg[:, mb, :], w1_sb[l][:, kb, mb, :],
                                         z2T[:, kb, :],
                                         start=(kb == 0), stop=False)
                    # fold b1 in as rank-1 term: b1_row^T @ ones_row
                    nc.tensor.matmul(pg[:, mb, :], b1_sb[l][:, mb, :],
                                     ones1_sb[:, :], start=False, stop=True)
                g = act.tile([128, NFB, B_LOC], F32, tag="g", name=f"g{l}")
                nc.scalar.activation(out=g[:, :, :], in_=pg[:, :, :],
                                     func=AF.Gelu_apprx_tanh, bias=0.0, scale=1.0)
                po2 = ps.tile([B_LOC, D], F32, tag="po", name=f"po_m{l}")
                for fb in range(NFB):
                    nc.tensor.matmul(po2[:, :], g[:, fb, :], w2_sb[l][:, fb, :],
                                     start=(fb == 0), stop=False)
                nc.tensor.matmul(po2[:, :], ones1_sb[:, :], b2_sb[l][:, :],
                                 start=False, stop=True)
                nc.vector.tensor_add(out=h_sb[:, :], in0=h_sb[:, :], in1=po2[:, :])

            # --- classifier head: out = h @ w_cls + b_cls ---
            hT = lnp.tile([128, NKB, B_LOC], F32, tag="zT", name="hT_cls")
            for kb in range(NKB):
                pt = ps.tile([128, B_LOC], F32, tag="pt", name=f"pt_cls{kb}")
                nc.tensor.transpose(pt[:, :], h_sb[:, kb * 128:(kb + 1) * 128],
                                    id2_sb[:, :])
                nc.scalar.copy(out=hT[:, kb, :], in_=pt[:, :])
            pc = ps.tile([B_LOC, C], F32, tag="pc", name="pc")
            for kb in range(NKB):
                nc.tensor.matmul(pc[:, :], hT[:, kb, :], wcls_sb[:, kb, :],
                                 start=(kb == 0), stop=False)
            nc.tensor.matmul(pc[:, :], ones1_sb[:, :], bcls_sb[:, :],
                             start=False, stop=True)
            out_sb = act.tile([B_LOC, C], F32, bufs=1)
            nc.vector.tensor_copy(out=out_sb, in_=pc[:, :])
            nc.sync.dma_start(out=out_d[:, :], in_=out_sb)

    nc.finalize()
    return nc


def _prep_weights(inputs):
    """Fold LN scale/bias into the weights (float64 on host) and lay tensors
    out partition-major for clean DMA."""
    f64 = lambda k: np.asarray(inputs[k], np.float64)
    ln1_s, ln1_b = f64('ln1_s'), f64('ln1_b')
    ln2_s, ln2_b = f64('ln2_s'), f64('ln2_b')
    wv, wo, bo = f64('wv'), f64('wo'), f64('bo')
    w1, b1 = f64('w1'), f64('b1')
    w2, b2 = f64('w2'), f64('b2')
    w_cls, b_cls = f64('w_cls'), f64('b_cls')

    wvo = np.zeros((L, 128, NKB, D), np.float32)
    bvo = np.zeros((L, 1, D), np.float32)
    w1b = np.zeros((L, 128, NKB, NFB, 128), np.float32)
    b1r = np.zeros((L, 1, NFB, 128), np.float32)
    w2b = np.zeros((L, 128, NFB, D), np.float32)
    b2r = np.zeros((L, 1, D), np.float32)
    for l in range(L):
        Wvo = (ln1_s[l][:, None] * wv[l]) @ wo[l]            # [D, D]
        Bvo = (ln1_b[l] @ wv[l]) @ wo[l] + bo[l]             # [D]
        W1p = ln2_s[l][:, None] * w1[l]                      # [D, FF]
        B1p = ln2_b[l] @ w1[l] + b1[l]                       # [FF]
        for kb in range(NKB):
            wvo[l, :, kb, :] = Wvo[kb * 128:(kb + 1) * 128, :]
            for mb in range(NFB):
                w1b[l, :, kb, mb, :] = W1p[kb * 128:(kb + 1) * 128,
                                           mb * 128:(mb + 1) * 128]
        for fb in range(NFB):
            w2b[l, :, fb, :] = w2[l][fb * 128:(fb + 1) * 128, :]
        bvo[l, 0, :] = Bvo
        b1r[l, 0, :, :] = B1p.reshape(NFB, 128)
        b2r[l, 0, :] = b2[l]

    wclsb = np.zeros((128, NKB, C), np.float32)
    for kb in range(NKB):
        wclsb[:, kb, :] = w_cls[kb * 128:(kb + 1) * 128, :]
    bclsr = b_cls.astype(np.float32).reshape(1, C)

    return {
        'wvo': wvo, 'bvo': bvo, 'w1b': w1b, 'b1r': b1r,
        'w2b': w2b, 'b2r': b2r, 'wclsb': wclsb, 'bclsr': bclsr,
        'id2': np.eye(B_LOC, dtype=np.float32),
        'ones1': np.ones((1, B_LOC), np.float32),
    }


def kernel(**inputs) -> np.ndarray:
    x = np.asarray(inputs['x'])
    emb_tok = np.asarray(inputs['emb_tok'], np.float32)
    emb_pos = np.asarray(inputs['emb_pos'], np.float32)

    # Token-0 residual stream per batch row: [B, D]
    h0 = emb_tok[x[:, 0]] + emb_pos[0][None, :]

    wmap = _prep_weights(inputs)

    if 'nc' not in _CACHE:
        _CACHE['nc'] = _build_nc()
    nc = _CACHE['nc']

    in_maps = []
    for c in range(N_CORES):
        m = dict(wmap)
        m['h0'] = np.ascontiguousarray(h0[c * B_LOC:(c + 1) * B_LOC])
        in_maps.append(m)

    res = run_bass_kernel_spmd(nc, in_maps, core_ids=list(range(N_CORES)))
    out = np.concatenate([res.results[c]['out'] for c in range(N_CORES)], axis=0)
    return out.astype(np.float32)


if __name__ == "__main__":
    import reference
    ins = reference.setup_inputs()
    ins = {k: np.asarray(v) for k, v in ins.items()}
    got = kernel(**ins)
    want = np.asarray(reference.reference(**ins))
    rel = np.abs(got - want).max() / np.abs(want).max()
    print("Relative error:", rel)
